# revision 18
# baseline (speedup 1.0000x reference)
"""Trainium2 Bass kernel for nn_DetectionLoss (YOLO-style detection loss).

Pure data-parallel over batch: 8 cores x 4096 samples.

Per-core decomposition (everything per-sample, samples on partitions):
  loss_sum = 0.5 * sum_all softplus(po)                                (dense)
           + sum_assigned [ sp(-po) - 0.5*sp(po) ]                     (dense, mask)
           + 5 * sum_assigned smoothL1(pb - t)                         (dense)
           + 2 * sum_assigned cw * (logsumexp(pc) - pc[lbl])           (dense)
  total    = loss_sum / max(num_pos, 1)                                (host)

The sparse->dense connection uses the GPSIMD `local_scatter` custom op:
each partition independently scatters its targets' channel values
(mask, class-weight one-hot x3, tx, ty, tw, th as fp16) into dense
per-cell grids (Qc*98 cells per partition).  Duplicate cell assignments
are pre-resolved on DVE ("is there a later valid target with the same
cell" - matches jax scatter last-write-wins); losers/invalid targets get
index -1 which local_scatter ignores.  softplus(x) = -ln(sigmoid(-x)).
Each core emits per-partition partial sums; the host combines.
"""
import sys

sys.path.insert(0, "/opt/trn_rl_repo")

import numpy as np

import concourse.bass as bass
import concourse.bacc as bacc
import concourse.tile as tile
from concourse import mybir
from concourse.bass_utils import run_bass_kernel_spmd

F32 = mybir.dt.float32
F16 = mybir.dt.float16
I32 = mybir.dt.int32
I16 = mybir.dt.int16
BF16 = mybir.dt.bfloat16
ALU = mybir.AluOpType
ACTF = mybir.ActivationFunctionType
AX = mybir.AxisListType

G = 7
A = 2
C = 3
NCELL = G * G * A  # 98
ROW = 5 + C        # 8
M = 20
P = 128
N_CORES = 8
L_COORD, L_OBJ, L_NOOBJ, L_CLS = 5.0, 1.0, 0.5, 2.0

ANCHORS = np.array([[0.971, 1.7338], [3.4579, 5.1653]], dtype=np.float32)
CLASS_WEIGHTS = np.array([1.0, 4.9, 4.8], dtype=np.float32)

NPART = 16  # partials columns per half


def _ap(t, offset_delta, dims):
    """Custom AP over tile/AP t: keep partition dim, replace free dims."""
    base = t[:] if not isinstance(t, bass.AP) else t
    return bass.AP(base.tensor, base.offset + offset_delta, [base.ap[0]] + dims)


def build_program(Q, halves=2):
    """One-core SPMD program. B_core = 128*Q samples."""
    Bc = P * Q
    assert Q % halves == 0
    Qc = Q // halves          # sample-groups per half (per partition)
    QM = Qc * M               # targets per partition per half
    ND = Qc * NCELL           # dense cells per partition per half
    assert ND * 32 < 2 ** 16  # local_scatter num_elems limit
    nc = bacc.Bacc("TRN2", target_bir_lowering=False)

    preds = nc.dram_tensor("preds", [Bc * NCELL, ROW], F32, kind="ExternalInput")
    boxes = nc.dram_tensor("boxes", [Bc, M, 4], F32, kind="ExternalInput")
    labels = nc.dram_tensor("labels", [Bc, M], I32, kind="ExternalInput")
    nobj = nc.dram_tensor("nobj", [Bc], I32, kind="ExternalInput")
    out_part = nc.dram_tensor("partials", [P, NPART * halves], F32,
                              kind="ExternalOutput")

    a0w, a0h = float(ANCHORS[0, 0]), float(ANCHORS[0, 1])
    a1w, a1h = float(ANCHORS[1, 0]), float(ANCHORS[1, 1])
    lw0 = float(np.log(np.float32(a0w) + np.float32(1e-6)))
    lw1 = float(np.log(np.float32(a1w) + np.float32(1e-6)))
    lh0 = float(np.log(np.float32(a0h) + np.float32(1e-6)))
    lh1 = float(np.log(np.float32(a1h) + np.float32(1e-6)))
    w0, w1, w2 = [float(x) for x in CLASS_WEIGHTS]

    V = nc.vector
    S = nc.scalar
    GP = nc.gpsimd

    boxes_r = boxes[:].rearrange("(p q) m c -> p (q m c)", p=P)
    labels_r = labels[:].rearrange("(p q) m -> p (q m)", p=P)
    nobj_r = nobj[:].rearrange("(p q) -> p q", p=P)
    preds_r = preds[:].rearrange("(p r) h -> p (r h)", p=P)

    with tile.TileContext(nc) as tc:
        with (
            tc.tile_pool(name="const", bufs=1) as const,
            tc.tile_pool(name="work", bufs=1) as work,
        ):
            def ct(name, shape, dtype=F32):
                return const.tile(shape, dtype, name=name, tag=name)

            def mk(name, shape, dtype=F32, bufs=1, pool=None):
                del bufs
                return (pool or work).tile(shape, dtype, name=name, tag=name,
                                           bufs=1)

            # ---------------- constants ----------------
            iota_m_i = ct("iota_m_i", [P, QM], I32)
            GP.iota(iota_m_i[:], pattern=[[0, Qc], [1, M]], base=0,
                    channel_multiplier=0)
            iota_m = ct("iota_m", [P, QM])
            V.tensor_copy(iota_m[:], iota_m_i[:])
            junk = ct("junk", [P, QM])
            V.tensor_scalar_add(junk[:], iota_m[:], 100.0)
            # NUT[m, m'] = 1.0 if m' <= m else 0.0 (m-major layout)
            nut_i = ct("nut_i", [P, M * M], I32)
            GP.iota(nut_i[:], pattern=[[-1, M], [1, M]], base=0,
                    channel_multiplier=0)
            nut = ct("nut", [P, M * M], BF16)
            V.tensor_scalar(nut[:], nut_i[:], 0, None, op0=ALU.is_le)
            # q*NCELL per (q, m): dense cell base within the partition
            q98_i = ct("q98_i", [P, QM], I32)
            GP.iota(q98_i[:], pattern=[[NCELL, Qc], [0, M]], base=0,
                    channel_multiplier=0)
            q98 = ct("q98", [P, QM])
            V.tensor_copy(q98[:], q98_i[:])
            ones16 = ct("ones16", [P, QM], F16)
            V.memset(ones16[:], 1.0)

            partials = ct("partials", [P, NPART * 2])
            V.memset(partials[:], 0.0)

            # ---------------- per-half pipeline ----------------
            for h in range(2):
                def col(i):
                    return partials[:, h * NPART + i:h * NPART + i + 1]

                cS = h * Qc * M
                bS = h * Qc * M * 4

                # ---- loads ----
                Tb = mk("Tb", [P, QM * 4])
                nc.sync.dma_start(out=Tb[:], in_=boxes_r[:, bS:bS + QM * 4])
                Tl_i = mk("Tl_i", [P, QM], I32)
                nc.sync.dma_start(out=Tl_i[:], in_=labels_r[:, cS:cS + QM])
                Tn_i = mk("Tn_i", [P, Qc], I32)
                nc.sync.dma_start(out=Tn_i[:],
                                  in_=nobj_r[:, h * Qc:(h + 1) * Qc])
                # full predictions for this half, cast to bf16 during DMA
                PR = mk("PR", [P, ND * ROW], BF16)
                GP.dma_start(out=PR[:],
                             in_=preds_r[:, h * ND * ROW:(h + 1) * ND * ROW])
                po_v = _ap(PR, 0, [[ROW, ND]])
                pc_v = _ap(PR, 5, [[ROW, ND], [1, C]])

                lblf = mk("lblf", [P, QM], bufs=2)
                V.tensor_copy(lblf[:], Tl_i[:])
                nobjf = mk("nobjf", [P, Qc], bufs=2)
                V.tensor_copy(nobjf[:], Tn_i[:])

                x1 = _ap(Tb, 0, [[4, QM]])
                y1 = _ap(Tb, 1, [[4, QM]])
                x2 = _ap(Tb, 2, [[4, QM]])
                y2 = _ap(Tb, 3, [[4, QM]])

                # ---- per-target quantities ----
                CXG = mk("CXG", [P, QM], bufs=2)
                V.tensor_tensor(CXG[:], x1, x2, op=ALU.add)
                V.tensor_scalar_mul(CXG[:], CXG[:], 0.5 * G)
                CYG = mk("CYG", [P, QM], bufs=2)
                V.tensor_tensor(CYG[:], y1, y2, op=ALU.add)
                V.tensor_scalar_mul(CYG[:], CYG[:], 0.5 * G)

                # floor via compare chain (values in [0, 7))
                GJ = mk("GJ", [P, QM], bufs=2)
                V.tensor_scalar(GJ[:], CXG[:], 1.0, None, op0=ALU.is_ge)
                for k in range(2, G):
                    V.scalar_tensor_tensor(GJ[:], CXG[:], float(k), GJ[:],
                                           op0=ALU.is_ge, op1=ALU.add)
                GI = mk("GI", [P, QM], bufs=2)
                V.tensor_scalar(GI[:], CYG[:], 1.0, None, op0=ALU.is_ge)
                for k in range(2, G):
                    V.scalar_tensor_tensor(GI[:], CYG[:], float(k), GI[:],
                                           op0=ALU.is_ge, op1=ALU.add)

                # tx, ty (fp16 contiguous, ready for scatter)
                TX = mk("TX", [P, QM], F16, bufs=2)
                V.tensor_tensor(TX[:], CXG[:], GJ[:], op=ALU.subtract)
                TY = mk("TY", [P, QM], F16, bufs=2)
                V.tensor_tensor(TY[:], CYG[:], GI[:], op=ALU.subtract)

                WG = mk("WG", [P, QM], bufs=2)
                V.tensor_tensor(WG[:], x2, x1, op=ALU.subtract)
                V.tensor_scalar_mul(WG[:], WG[:], float(G))
                HG = mk("HG", [P, QM], bufs=2)
                V.tensor_tensor(HG[:], y2, y1, op=ALU.subtract)
                V.tensor_scalar_mul(HG[:], HG[:], float(G))

                VALID = mk("VALID", [P, QM], bufs=2)
                V.tensor_tensor(VALID[:], _ap(nobjf, 0, [[1, Qc], [0, M]]),
                                iota_m[:], op=ALU.is_gt)

                AR = mk("AR", [P, QM], bufs=2)
                V.tensor_tensor(AR[:], WG[:], HG[:], op=ALU.mult)
                T1 = mk("T1", [P, QM], bufs=2)
                T2 = mk("T2", [P, QM], bufs=2)
                # anchor 0
                V.tensor_scalar_min(T1[:], WG[:], a0w)
                V.tensor_scalar_min(T2[:], HG[:], a0h)
                I0 = mk("I0", [P, QM], bufs=2)
                V.tensor_tensor(I0[:], T1[:], T2[:], op=ALU.mult)
                U0 = mk("U0", [P, QM], bufs=2)
                V.tensor_scalar_add(U0[:], AR[:], a0w * a0h + 1e-6)
                V.tensor_tensor(U0[:], U0[:], I0[:], op=ALU.subtract)
                # anchor 1
                V.tensor_scalar_min(T1[:], WG[:], a1w)
                V.tensor_scalar_min(T2[:], HG[:], a1h)
                I1 = mk("I1", [P, QM], bufs=2)
                V.tensor_tensor(I1[:], T1[:], T2[:], op=ALU.mult)
                U1 = mk("U1", [P, QM], bufs=2)
                V.tensor_scalar_add(U1[:], AR[:], a1w * a1h + 1e-6)
                V.tensor_tensor(U1[:], U1[:], I1[:], op=ALU.subtract)
                # argmax via cross-multiply (strict > matches first-max)
                V.tensor_tensor(T1[:], I1[:], U0[:], op=ALU.mult)
                V.tensor_tensor(T2[:], I0[:], U1[:], op=ALU.mult)
                BEST = mk("BEST", [P, QM], bufs=2)
                V.tensor_tensor(BEST[:], T1[:], T2[:], op=ALU.is_gt)

                # tw/th (fp16 contiguous)
                V.tensor_scalar(T1[:], BEST[:], lw1 - lw0, lw0,
                                op0=ALU.mult, op1=ALU.add)
                V.tensor_scalar_max(T2[:], WG[:], 0.01)
                LN1 = mk("LN1", [P, QM], bufs=2)
                S.activation(LN1[:], T2[:], ACTF.Ln)
                TW = mk("TW", [P, QM], F16, bufs=2)
                V.tensor_tensor(TW[:], LN1[:], T1[:], op=ALU.subtract)
                V.tensor_scalar(T1[:], BEST[:], lh1 - lh0, lh0,
                                op0=ALU.mult, op1=ALU.add)
                V.tensor_scalar_max(T2[:], HG[:], 0.01)
                LN2 = mk("LN2", [P, QM], bufs=2)
                S.activation(LN2[:], T2[:], ACTF.Ln)
                TH = mk("TH", [P, QM], F16, bufs=2)
                V.tensor_tensor(TH[:], LN2[:], T1[:], op=ALU.subtract)

                FLAT = mk("FLAT", [P, QM], bufs=2)
                V.scalar_tensor_tensor(FLAT[:], GI[:], float(G), GJ[:],
                                       op0=ALU.mult, op1=ALU.add)
                V.scalar_tensor_tensor(FLAT[:], FLAT[:], float(A), BEST[:],
                                       op0=ALU.mult, op1=ALU.add)

                FENC = mk("FENC", [P, QM], bufs=2)
                V.tensor_copy(FENC[:], junk[:])
                VALID_I = mk("VALID_I", [P, QM], I32, bufs=2)
                V.tensor_copy(VALID_I[:], VALID[:])
                V.copy_predicated(FENC[:], VALID_I[:], FLAT[:])

                # ---- owner detection (last valid wins) ----
                EQ = mk("EQ", [P, Qc, M, M], BF16, bufs=1)
                fencA = _ap(FENC, 0, [[M, Qc], [1, M], [0, M]])
                fencB = _ap(FENC, 0, [[M, Qc], [0, M], [1, M]])
                V.tensor_tensor(EQ[:], fencA, fencB, op=ALU.is_equal)
                V.scalar_tensor_tensor(
                    EQ[:], EQ[:], 1.0,
                    _ap(nut, 0, [[0, Qc], [M, M], [1, M]]),
                    op0=ALU.mult, op1=ALU.subtract)
                DUP = mk("DUP", [P, QM], bufs=2)
                V.tensor_reduce(DUP[:], EQ[:], axis=AX.X, op=ALU.max)
                OWNER = mk("OWNER", [P, QM], bufs=2)
                V.scalar_tensor_tensor(OWNER[:], DUP[:], 0.0, VALID[:],
                                       op0=ALU.is_le, op1=ALU.mult,
                                       accum_out=col(10))

                # ---- scatter indices: owner ? q*98+flat : -1 ----
                CIDX = mk("CIDX", [P, QM], bufs=2)
                V.tensor_tensor(CIDX[:], FLAT[:], q98[:], op=ALU.add)
                V.tensor_scalar_add(CIDX[:], CIDX[:], 1.0)
                V.tensor_tensor(CIDX[:], CIDX[:], OWNER[:], op=ALU.mult)
                V.tensor_scalar_add(CIDX[:], CIDX[:], -1.0)
                IDX16 = mk("IDX16", [P, QM], I16, bufs=2)
                V.tensor_copy(IDX16[:], CIDX[:])

                # ---- class-weight one-hot channels (fp16) ----
                WOH = []
                for c in range(C):
                    wc = mk(f"WOH{c}", [P, QM], F16, bufs=2)
                    V.tensor_scalar(wc[:], lblf[:], float(c),
                                    [w0, w1, w2][c],
                                    op0=ALU.is_equal, op1=ALU.mult)
                    WOH.append(wc)

                # ---- local scatters into dense grids ----
                def scat(name, data_t):
                    g = mk(name, [P, ND], F16)
                    GP.local_scatter(out_ap=g[:], data_ap=data_t[:],
                                     idxs_ap=IDX16[:], channels=P,
                                     num_elems=ND, num_idxs=QM)
                    return g

                MKD = scat("MKD", ones16)
                W0D = scat("W0D", WOH[0])
                W1D = scat("W1D", WOH[1])
                W2D = scat("W2D", WOH[2])
                TXD = scat("TXD", TX)
                TYD = scat("TYD", TY)
                TWD = scat("TWD", TW)
                THD = scat("THD", TH)

                # ---- dense: obj / noobj ----
                # LB = ln(sig(-po)) = -sp(po); LA = ln(sig(po)) = -sp(-po)
                SGD = mk("SGD", [P, ND], BF16, bufs=2)
                S.activation(SGD[:], po_v, ACTF.Sigmoid, scale=-1.0)
                LBD = mk("LBD", [P, ND], BF16, bufs=2)
                S.activation(LBD[:], SGD[:], ACTF.Ln, accum_out=col(0))
                S.activation(SGD[:], po_v, ACTF.Sigmoid)
                LAD = mk("LAD", [P, ND], BF16, bufs=2)
                S.activation(LAD[:], SGD[:], ACTF.Ln)
                # objt = 0.5*LB - LA  (= sp(-po) - 0.5*sp(po))
                OBD = mk("OBD", [P, ND], bufs=2)
                V.scalar_tensor_tensor(OBD[:], LBD[:], L_NOOBJ, LAD[:],
                                       op0=ALU.mult, op1=ALU.subtract)
                V.scalar_tensor_tensor(OBD[:], OBD[:], 1.0, MKD[:],
                                       op0=ALU.mult, op1=ALU.mult,
                                       accum_out=col(1))

                # ---- dense: smooth L1 ----
                DD = mk("DD", [P, ND], bufs=2)
                MND = mk("MND", [P, ND], bufs=2)
                for ci, TD in enumerate([TXD, TYD, TWD, THD]):
                    pb_c = _ap(PR, 1 + ci, [[ROW, ND]])
                    V.tensor_tensor(DD[:], pb_c, TD[:], op=ALU.subtract)
                    V.tensor_tensor(DD[:], DD[:], MKD[:], op=ALU.mult)
                    ddi = DD[:].bitcast(I32)
                    V.tensor_scalar(ddi, ddi, 0x7FFFFFFF, None,
                                    op0=ALU.bitwise_and)
                    V.tensor_scalar_min(MND[:], DD[:], 1.0)
                    V.scalar_tensor_tensor(DD[:], MND[:], -0.5, DD[:],
                                           op0=ALU.mult, op1=ALU.add)
                    V.scalar_tensor_tensor(DD[:], MND[:], 1.0, DD[:],
                                           op0=ALU.mult, op1=ALU.mult,
                                           accum_out=col(2 + ci))

                # ---- dense: weighted cross entropy ----
                EZD = mk("EZD", [P, ND, C], BF16, bufs=2)
                S.activation(EZD[:], pc_v, ACTF.Exp)
                ZD = mk("ZD", [P, ND], bufs=2)
                V.tensor_reduce(ZD[:], EZD[:], axis=AX.X, op=ALU.add)
                LZD = mk("LZD", [P, ND], BF16, bufs=2)
                S.activation(LZD[:], ZD[:], ACTF.Ln)
                CWD = mk("CWD", [P, ND], bufs=2)
                V.tensor_tensor(CWD[:], W0D[:], W1D[:], op=ALU.add)
                V.tensor_tensor(CWD[:], CWD[:], W2D[:], op=ALU.add)
                V.scalar_tensor_tensor(CWD[:], CWD[:], 1.0, LZD[:],
                                       op0=ALU.mult, op1=ALU.mult,
                                       accum_out=col(6))
                LGT = mk("LGT", [P, ND], bufs=2)
                for c, WD in enumerate([W0D, W1D, W2D]):
                    pc_c = _ap(PR, 5 + c, [[ROW, ND]])
                    V.scalar_tensor_tensor(LGT[:], WD[:], 1.0, pc_c,
                                           op0=ALU.mult, op1=ALU.mult,
                                           accum_out=col(7 + c))

            nc.sync.dma_start(out=out_part[:], in_=partials[:])

    nc.finalize()
    return nc


_CACHE = {}


def _get_program(Q):
    if Q not in _CACHE:
        _CACHE[Q] = build_program(Q)
    return _CACHE[Q]


def shard_inputs(predictions, target_boxes, target_labels, num_objs):
    B = predictions.shape[0]
    Bc = B // N_CORES
    preds = np.ascontiguousarray(predictions, dtype=np.float32).reshape(
        N_CORES, Bc * NCELL, ROW)
    boxes = np.ascontiguousarray(target_boxes, dtype=np.float32).reshape(
        N_CORES, Bc, M, 4)
    labels = np.ascontiguousarray(target_labels, dtype=np.int32).reshape(
        N_CORES, Bc, M)
    nobj = np.ascontiguousarray(num_objs, dtype=np.int32).reshape(N_CORES, Bc)
    return [
        dict(preds=preds[i], boxes=boxes[i], labels=labels[i], nobj=nobj[i])
        for i in range(N_CORES)
    ]


def combine_partials(parts, halves=2):
    """parts: list of (P, NPART*halves) arrays."""
    s = np.zeros(NPART, np.float64)
    for p in parts:
        p = p.astype(np.float64)
        for h in range(halves):
            s += p[:, h * NPART:(h + 1) * NPART].sum(axis=0)
    neg_sp_all = s[0]          # sum ln(sig(-po)) over all cells = -sum sp(po)
    obj_a = s[1]               # sum mask*(sp(-po) - 0.5 sp(po))
    sl1 = s[2] + s[3] + s[4] + s[5]
    ce_lz = s[6]
    ce_logit = s[7] + s[8] + s[9]
    npos = s[10]
    loss_sum = (-L_NOOBJ * neg_sp_all + obj_a + L_COORD * sl1
                + L_CLS * (ce_lz - ce_logit))
    total = loss_sum / max(npos, 1.0)
    return np.float32(total)


def kernel(predictions, target_boxes, target_labels, num_objs,
           anchors=None, class_weights=None, **_):
    B = predictions.shape[0]
    Q = B // (N_CORES * P)
    nc = _get_program(Q)
    in_maps = shard_inputs(predictions, target_boxes, target_labels, num_objs)
    res = run_bass_kernel_spmd(nc, in_maps, core_ids=list(range(N_CORES)))
    return combine_partials([r["partials"] for r in res.results])
